# revision 11
# baseline (speedup 1.0000x reference)
"""Multi-head self-attention TRN2 kernel.

Full inputs -> shard over 8 NeuronCores as (batch b, head-group g):
core c = 2*b + g handles batch b and heads 8g..8g+7 (tensor parallel over
heads within a batch entry). Each core computes its heads' contribution to
the output projection; the host sums the two partials per batch and adds
proj bias.

Per-core pipeline (all matmuls bf16 with fp32 PSUM accumulation):
  x^T   via transposing DMA                      [C=1024, T=2048]
  Q^T,K^T = (w.T as lhsT) @ x^T  (+bias, DVE)    [512, T] col-major
  V     = (x^T as lhsT) @ wv (+bias via K=1 mm)  [T, 512] + ones col
  per head-pair hp, t-chunk of 1024, s-tile of 128:
    S^T chunk = K^T.T @ Q^T      (K=64 contraction, 2 heads row-tiled)
    P^T = exp(S^T/8)             ScalarE from PSUM, bf16 out, N=1024
    O  += P^T.T @ V_aug          (per 128-t subtile, N=65; col 64 = rowsum)
  normalize O by 1/rowsum (per-partition scalar), DMA-transpose -> O^T
  partial = O^T.T @ pw           [T, 1024] fp32 -> DRAM
"""

import numpy as np
import ml_dtypes
from contextlib import ExitStack

import concourse.bass as bass
import concourse.bacc as bacc
import concourse.tile as tile
from concourse import mybir
from concourse.bass_utils import run_bass_kernel_spmd

BF16 = mybir.dt.bfloat16
F32 = mybir.dt.float32
bf16 = ml_dtypes.bfloat16

P = 128
C = 1024          # hidden
HG = 8            # heads per core
D = 64            # head dim
DG = HG * D       # 512, per-core qkv width
N_CORES = 8
FULL_T = 2048
SCALE = D ** -0.5


def build_kernel(T=FULL_T):
    nc = bacc.Bacc(
        "TRN2", target_bir_lowering=False, debug=False, num_devices=N_CORES
    )
    x = nc.dram_tensor("x", [T, C], BF16, kind="ExternalInput").ap()
    wq = nc.dram_tensor("wq", [P, C // P, DG], BF16, kind="ExternalInput").ap()
    wk = nc.dram_tensor("wk", [P, C // P, DG], BF16, kind="ExternalInput").ap()
    wv = nc.dram_tensor("wv", [P, C // P, DG], BF16, kind="ExternalInput").ap()
    # cols 0..3 = q bias per col-tile, 4..7 = k bias
    bqk = nc.dram_tensor("bqk", [P, 8], F32, kind="ExternalInput").ap()
    bv = nc.dram_tensor("bv", [1, DG], BF16, kind="ExternalInput").ap()
    pw = nc.dram_tensor("pw", [P, DG // P, C], BF16, kind="ExternalInput").ap()
    partial = nc.dram_tensor("partial", [T, C], F32, kind="ExternalOutput").ap()

    CT = C // P           # 8 contraction tiles over hidden
    TT = T // P           # t/s tiles of 128
    TCH = min(1024, T)    # t chunk width for attention (exp granularity)
    NCH = T // TCH        # number of t chunks
    SW = min(512, TCH)    # matmul moving-dim width (PSUM bank limit)
    NSW = TCH // SW       # sub-chunks per chunk
    Q8 = TCH // P         # 128-t subtiles per chunk
    KT4 = DG // P         # 4 col-tiles of Q^T/K^T/O^T

    with tile.TileContext(nc) as tc, ExitStack() as ctx:
        sb = ctx.enter_context(tc.tile_pool(name="sb", bufs=1))
        pdram = ctx.enter_context(tc.tile_pool(name="pdram", bufs=4, space="DRAM"))
        pon = ctx.enter_context(tc.tile_pool(name="pon", bufs=3))
        ppb = ctx.enter_context(tc.tile_pool(name="ppb", bufs=6))
        pout = ctx.enter_context(tc.tile_pool(name="pout", bufs=2))
        pp = ctx.enter_context(tc.tile_pool(name="pp", bufs=1, space="PSUM"))

        def o_tag(hx):
            return ("oA", "oB")[hx]

        # persistent SBUF tensors
        xT = sb.tile([P, CT, T], BF16)
        wq_s = sb.tile([P, CT, DG], BF16)
        wk_s = sb.tile([P, CT, DG], BF16)
        wv_s = sb.tile([P, CT, DG], BF16)
        pw_s = sb.tile([P, KT4, C], BF16)
        bqk_s = sb.tile([P, 8], F32)
        bv_s = sb.tile([1, DG], BF16)
        ones_s = sb.tile([1, P], BF16)
        QT = sb.tile([P, KT4, T], BF16)
        KTt = sb.tile([P, KT4, T], BF16)
        V = sb.tile([P, TT, HG, 65], BF16)
        OT = sb.tile([P, KT4, T], BF16)

        # ---- loads ----
        nc.scalar.dma_start(out=wq_s, in_=wq)
        nc.scalar.dma_start(out=wk_s, in_=wk)
        nc.scalar.dma_start(out=wv_s, in_=wv)
        nc.scalar.dma_start(out=pw_s, in_=pw)
        nc.scalar.dma_start(out=bqk_s, in_=bqk)
        nc.scalar.dma_start(out=bv_s, in_=bv)
        nc.vector.memset(ones_s, 1.0)
        nc.vector.memset(V[:, :, :, 64:65], 1.0)
        # x^T: transposing DMA, one column-block of x at a time
        for ct in range(CT):
            nc.sync.dma_start(
                out=xT[:, ct, :], in_=x[:, ct * P : (ct + 1) * P], transpose=True
            )

        # ---- V (natural layout, bias via K=1 matmul, ones col preset) ----
        for tt in range(TT):
            ps = pp.tile([P, DG], F32, tag=o_tag(tt % 2), name="psv")
            for ct in range(CT):
                nc.tensor.matmul(
                    ps,
                    lhsT=xT[:, ct, tt * P : (tt + 1) * P],
                    rhs=wv_s[:, ct, :],
                    start=(ct == 0),
                    stop=False,
                )
            nc.tensor.matmul(ps, lhsT=ones_s, rhs=bv_s, start=False, stop=True)
            nc.vector.tensor_copy(
                out=V[:, tt, :, 0:64],
                in_=ps.rearrange("p (h d) -> p h d", h=HG),
            )

        # ---- Q^T / K^T ----
        for wi, (w_s, QKT, boff) in enumerate(((wq_s, QT, 0), (wk_s, KTt, 4))):
            for i in range(KT4):
                for th in range(NCH):
                    ps = pp.tile([P, TCH], F32, tag=o_tag((i + th) % 2), name="ps")
                    for ct in range(CT):
                        for nh in range(NSW):
                            nc.tensor.matmul(
                                ps[:, nh * SW : (nh + 1) * SW],
                                lhsT=w_s[:, ct, i * P : (i + 1) * P],
                                rhs=xT[
                                    :, ct, th * TCH + nh * SW : th * TCH + (nh + 1) * SW
                                ],
                                start=(ct == 0),
                                stop=(ct == CT - 1),
                            )
                    nc.vector.tensor_scalar_add(
                        QKT[:, i, th * TCH : (th + 1) * TCH],
                        ps,
                        bqk_s[:, boff + i : boff + i + 1],
                    )

        # ---- attention ----
        for hp in range(KT4):          # head pair = col-tile of QT/KT
            for th in range(NCH):      # t chunk of TCH
                # O_aug^T accumulators [65, TCH]: rows 0..63 = O^T, row 64 =
                # softmax denominator (from the ones column of V_aug)
                o_t = [
                    pp.tile([65, TCH], F32, tag=o_tag(hx), name="o_t")
                    for hx in range(2)
                ]
                for st in range(TT):   # s tile of 128
                    # both heads' S^T in one 4-bank tile -> one exp op;
                    # adjacent A/B matmuls overlap in distinct PE row-groups
                    s_ps = pp.tile([P, 2, TCH], F32, tag="sp", name="s_ps")
                    for nh in range(NSW):
                        for hx in range(2):
                            pr = slice(hx * 64, (hx + 1) * 64)
                            nc.tensor.matmul(
                                s_ps[:, hx, nh * SW : (nh + 1) * SW],
                                lhsT=KTt[pr, hp, st * P : (st + 1) * P],
                                rhs=QT[
                                    pr,
                                    hp,
                                    th * TCH + nh * SW : th * TCH + (nh + 1) * SW,
                                ],
                                start=True,
                                stop=True,
                            )
                    pb = ppb.tile([P, 2, TCH], BF16, tag="p", name="pb")
                    nc.scalar.activation(
                        out=pb,
                        in_=s_ps,
                        func=mybir.ActivationFunctionType.Exp,
                        scale=float(SCALE),
                    )
                    # O_aug^T += V_aug.T @ P^T  (V stationary, P streams)
                    for hx in range(2):
                        h = 2 * hp + hx
                        for nh in range(NSW):
                            nc.tensor.matmul(
                                o_t[hx][:, nh * SW : (nh + 1) * SW],
                                lhsT=V[:, st, h, :],
                                rhs=pb[:, hx, nh * SW : (nh + 1) * SW],
                                start=(st == 0),
                                stop=(st == TT - 1),
                            )
                # normalize: OT = O^T * (1/rowsum) with rowsum broadcast
                # across partitions via stride-0 DMA
                for hx in range(2):
                    # rowsum row -> DRAM bounce -> broadcast to 64 partitions
                    # (stride-0 partition reads are only legal from DRAM),
                    # then reciprocal across 64 lanes and scale O^T in place
                    rr = pon.tile([1, TCH], F32, tag="rr", name="rr")
                    nc.vector.tensor_copy(rr, o_t[hx][64:65, :])
                    rrd = pdram.tile([1, TCH], F32, tag="rrd", name="rrd")
                    nc.sync.dma_start(out=rrd, in_=rr)
                    rb = pon.tile([64, TCH], F32, tag="rb", name="rb")
                    nc.sync.dma_start(out=rb, in_=rrd.to_broadcast((64, TCH)))
                    nc.vector.reciprocal(rb, rb)
                    nc.vector.tensor_mul(
                        OT[hx * 64 : (hx + 1) * 64, hp, th * TCH : (th + 1) * TCH],
                        o_t[hx][0:64, :],
                        rb,
                    )

        # ---- projection ----
        for mt in range(TT):
            ps_p = pp.tile([P, C], F32, tag=o_tag(mt % 2), name="ps_p")
            for kk in range(KT4):
                for nh in range(C // 512):
                    nc.tensor.matmul(
                        ps_p[:, nh * 512 : (nh + 1) * 512],
                        lhsT=OT[:, kk, mt * P : (mt + 1) * P],
                        rhs=pw_s[:, kk, nh * 512 : (nh + 1) * 512],
                        start=(kk == 0),
                        stop=(kk == KT4 - 1),
                    )
            ot = pout.tile([P, C], F32, tag="ot", name="ot")
            nc.vector.tensor_copy(ot, ps_p)
            nc.sync.dma_start(out=partial[mt * P : (mt + 1) * P, :], in_=ot)

    nc.compile()
    return nc


def shard_inputs(x, qkv_w, qkv_b, proj_w, proj_b, T=FULL_T):
    """Build the 8 per-core input maps (host-side layout prep)."""
    x = np.asarray(x, dtype=np.float32)
    qkv_w = np.asarray(qkv_w, dtype=np.float32)
    qkv_b = np.asarray(qkv_b, dtype=np.float32)
    proj_w = np.asarray(proj_w, dtype=np.float32)
    in_maps = []
    for c in range(N_CORES):
        b, g = divmod(c, 2)
        sl = slice(g * DG, (g + 1) * DG)
        wqg = qkv_w[:, 0 * C + g * DG : 0 * C + (g + 1) * DG]
        wkg = qkv_w[:, 1 * C + g * DG : 1 * C + (g + 1) * DG]
        wvg = qkv_w[:, 2 * C + g * DG : 2 * C + (g + 1) * DG]
        bqg = qkv_b[0 * C + g * DG : 0 * C + (g + 1) * DG]
        bkg = qkv_b[1 * C + g * DG : 1 * C + (g + 1) * DG]
        bvg = qkv_b[2 * C + g * DG : 2 * C + (g + 1) * DG]
        pwg = proj_w[sl, :]

        def arr_w(w):  # [C, DG] -> [128, C//128, DG]
            return np.ascontiguousarray(
                w.reshape(C // P, P, DG).transpose(1, 0, 2)
            ).astype(bf16)

        bqk = np.ascontiguousarray(
            np.concatenate(
                [bqg.reshape(DG // P, P).T, bkg.reshape(DG // P, P).T], axis=1
            )
        ).astype(np.float32)
        in_maps.append(
            {
                "x": np.ascontiguousarray(x[b, :T]).astype(bf16),
                "wq": arr_w(wqg),
                "wk": arr_w(wkg),
                "wv": arr_w(wvg),
                "bqk": bqk,
                "bv": np.ascontiguousarray(bvg[None, :]).astype(bf16),
                "pw": np.ascontiguousarray(
                    pwg.reshape(DG // P, P, C).transpose(1, 0, 2)
                ).astype(bf16),
            }
        )
    return in_maps


def combine_outputs(results, proj_b, T=FULL_T):
    proj_b = np.asarray(proj_b, dtype=np.float32)
    out = np.empty((N_CORES // 2, T, C), np.float32)
    for b in range(N_CORES // 2):
        out[b] = (
            results[2 * b]["partial"] + results[2 * b + 1]["partial"] + proj_b
        )
    return out


_NC_CACHE = {}


def _get_nc(T=FULL_T):
    if T not in _NC_CACHE:
        _NC_CACHE[T] = build_kernel(T)
    return _NC_CACHE[T]


def run(x, qkv_w, qkv_b, proj_w, proj_b, trace=False):
    nc = _get_nc()
    in_maps = shard_inputs(x, qkv_w, qkv_b, proj_w, proj_b)
    res = run_bass_kernel_spmd(nc, in_maps, list(range(N_CORES)), trace=trace)
    return combine_outputs(res.results, proj_b), res


def kernel(x, qkv_w, qkv_b, proj_w, proj_b):
    out, _ = run(x, qkv_w, qkv_b, proj_w, proj_b)
    return out


# revision 16
# speedup vs baseline: 1.4581x; 1.4581x over previous
"""Multi-head self-attention TRN2 kernel.

Full inputs -> shard over 8 NeuronCores as (batch b, head-group g):
core c = 2*b + g handles batch b and heads 8g..8g+7 (tensor parallel over
heads within a batch entry). Each core computes its heads' contribution to
the output projection; the host sums the two partials per batch and adds
proj bias.

Per-core pipeline (all matmuls bf16 with fp32 PSUM accumulation):
  x^T   via transposing DMA                      [C=1024, T=2048]
  Q^T,K^T = (w.T as lhsT) @ x^T  (+bias, DVE)    [512, T] col-major
  V     = (x^T as lhsT) @ wv (+bias via K=1 mm)  [T, 512] + ones col
  per head-pair hp, t-chunk of 1024, s-tile of 128:
    S^T chunk = K^T.T @ Q^T      (K=64 contraction, 2 heads row-tiled)
    P^T = exp(S^T/8)             ScalarE from PSUM, bf16 out, N=1024
    O  += P^T.T @ V_aug          (per 128-t subtile, N=65; col 64 = rowsum)
  normalize O by 1/rowsum (per-partition scalar), DMA-transpose -> O^T
  partial = O^T.T @ pw           [T, 1024] fp32 -> DRAM
"""

import numpy as np
import ml_dtypes
from contextlib import ExitStack

import concourse.bass as bass
import concourse.bacc as bacc
import concourse.tile as tile
from concourse import mybir
from concourse.bass_utils import run_bass_kernel_spmd

BF16 = mybir.dt.bfloat16
F32 = mybir.dt.float32
bf16 = ml_dtypes.bfloat16

P = 128
C = 1024          # hidden
HG = 8            # heads per core
D = 64            # head dim
DG = HG * D       # 512, per-core qkv width
N_CORES = 8
FULL_T = 2048
SCALE = D ** -0.5


def build_kernel(T=FULL_T):
    nc = bacc.Bacc(
        "TRN2", target_bir_lowering=False, debug=False, num_devices=N_CORES
    )
    x = nc.dram_tensor("x", [T, C], BF16, kind="ExternalInput").ap()
    wq = nc.dram_tensor("wq", [P, C // P, DG], BF16, kind="ExternalInput").ap()
    wk = nc.dram_tensor("wk", [P, C // P, DG], BF16, kind="ExternalInput").ap()
    wv = nc.dram_tensor("wv", [P, C // P, DG], BF16, kind="ExternalInput").ap()
    # cols 0..3 = q bias per col-tile, 4..7 = k bias
    bqk = nc.dram_tensor("bqk", [P, 8], F32, kind="ExternalInput").ap()
    bv = nc.dram_tensor("bv", [1, DG], BF16, kind="ExternalInput").ap()
    pw = nc.dram_tensor("pw", [P, DG // P, C], BF16, kind="ExternalInput").ap()
    partial = nc.dram_tensor("partial", [C, T], F32, kind="ExternalOutput").ap()

    CT = C // P           # 8 contraction tiles over hidden
    TT = T // P           # t/s tiles of 128
    TCH = min(1024, T)    # t chunk width for attention (exp granularity)
    NCH = T // TCH        # number of t chunks
    SW = min(512, TCH)    # matmul moving-dim width (PSUM bank limit)
    NSW = TCH // SW       # sub-chunks per chunk
    Q8 = TCH // P         # 128-t subtiles per chunk
    KT4 = DG // P         # 4 col-tiles of Q^T/K^T/O^T

    with tile.TileContext(nc) as tc, ExitStack() as ctx:
        sb = ctx.enter_context(tc.tile_pool(name="sb", bufs=1))
        pdram = ctx.enter_context(tc.tile_pool(name="pdram", bufs=4, space="DRAM"))
        pon = ctx.enter_context(tc.tile_pool(name="pon", bufs=3))
        ppb = ctx.enter_context(tc.tile_pool(name="ppb", bufs=10))
        pout = ctx.enter_context(tc.tile_pool(name="pout", bufs=2))
        pp = ctx.enter_context(tc.tile_pool(name="pp", bufs=1, space="PSUM"))

        def o_tag(hx):
            return ("oA", "oB")[hx]

        # persistent SBUF tensors
        xT = sb.tile([P, CT, T], BF16)
        wq_s = sb.tile([P, CT, DG], BF16)
        wk_s = sb.tile([P, CT, DG], BF16)
        wv_s = sb.tile([P, CT, DG], BF16)
        pw_s = sb.tile([P, KT4, C], BF16)
        bqk_s = sb.tile([P, 8], F32)
        bv_s = sb.tile([1, DG], BF16)
        ones_s = sb.tile([1, P], BF16)
        QT = sb.tile([P, KT4, T], BF16)
        KTt = sb.tile([P, KT4, T], BF16)
        V = sb.tile([P, TT, HG, 65], BF16)
        OT = sb.tile([P, KT4, T], BF16)

        # ---- loads ----
        # all transposing DMAs first (xbar-mode flips serialize against
        # normal copies), split across the two HWDGE queues
        for ct in range(CT):
            eng = nc.sync if ct % 2 == 0 else nc.scalar
            eng.dma_start(
                out=xT[:, ct, :], in_=x[:, ct * P : (ct + 1) * P], transpose=True
            )
        nc.scalar.dma_start(out=wv_s, in_=wv)
        nc.scalar.dma_start(out=bv_s, in_=bv)
        nc.sync.dma_start(out=wq_s, in_=wq)
        nc.sync.dma_start(out=wk_s, in_=wk)
        nc.sync.dma_start(out=bqk_s, in_=bqk)
        nc.sync.dma_start(out=pw_s, in_=pw)
        nc.vector.memset(ones_s, 1.0)
        nc.vector.memset(V[:, :, :, 64:65], 1.0)

        # ---- V (natural layout, bias via K=1 matmul, ones col preset) ----
        for tt in range(TT):
            ps = pp.tile([P, DG], F32, tag=o_tag(tt % 2), name="psv")
            for ct in range(CT):
                nc.tensor.matmul(
                    ps,
                    lhsT=xT[:, ct, tt * P : (tt + 1) * P],
                    rhs=wv_s[:, ct, :],
                    start=(ct == 0),
                    stop=False,
                )
            nc.tensor.matmul(ps, lhsT=ones_s, rhs=bv_s, start=False, stop=True)
            nc.vector.tensor_copy(
                out=V[:, tt, :, 0:64],
                in_=ps.rearrange("p (h d) -> p h d", h=HG),
            )

        # ---- Q^T / K^T ----
        for wi, (w_s, QKT, boff) in enumerate(((wq_s, QT, 0), (wk_s, KTt, 4))):
            for i in range(KT4):
                for th in range(NCH):
                    ps = pp.tile([P, TCH], F32, tag=o_tag((i + th) % 2), name="ps")
                    for ct in range(CT):
                        for nh in range(NSW):
                            nc.tensor.matmul(
                                ps[:, nh * SW : (nh + 1) * SW],
                                lhsT=w_s[:, ct, i * P : (i + 1) * P],
                                rhs=xT[
                                    :, ct, th * TCH + nh * SW : th * TCH + (nh + 1) * SW
                                ],
                                start=(ct == 0),
                                stop=(ct == CT - 1),
                            )
                    nc.vector.tensor_scalar_add(
                        QKT[:, i, th * TCH : (th + 1) * TCH],
                        ps,
                        bqk_s[:, boff + i : boff + i + 1],
                    )

        # ---- attention ----
        # Software-pipelined: two head-streams (A/B) alternate on ScalarE;
        # while exp(B, st) runs, PE does PV(A, st) and S(A, st+1), so the
        # exp stream never waits. PSUM: sA, sB ([128, TCH] = 2 banks each,
        # bufs=1) + oA, oB ([65, TCH] accumulators) = 8 banks exactly.
        for hp in range(KT4):          # head pair = col-tile of QT/KT
            for th in range(NCH):      # t chunk of TCH
                o_t = [
                    pp.tile([65, TCH], F32, tag=o_tag(hx), name="o_t")
                    for hx in range(2)
                ]

                def s_mm(hx, st):
                    s_ps = pp.tile(
                        [P, TCH], F32, tag=("sA", "sB")[hx], name="s_ps"
                    )
                    pr = slice(hx * 64, (hx + 1) * 64)
                    for nh in range(NSW):
                        nc.tensor.matmul(
                            s_ps[:, nh * SW : (nh + 1) * SW],
                            lhsT=KTt[pr, hp, st * P : (st + 1) * P],
                            rhs=QT[
                                pr,
                                hp,
                                th * TCH + nh * SW : th * TCH + (nh + 1) * SW,
                            ],
                            start=True,
                            stop=True,
                        )
                    return s_ps

                s_cur = [s_mm(0, 0), s_mm(1, 0)]
                for st in range(TT):   # s tile of 128
                    for hx in range(2):
                        h = 2 * hp + hx
                        pb = ppb.tile([P, TCH], BF16, tag="p", name="pb")
                        nc.scalar.activation(
                            out=pb,
                            in_=s_cur[hx],
                            func=mybir.ActivationFunctionType.Exp,
                            scale=float(SCALE),
                        )
                        # O_aug^T += V_aug.T @ P^T (V stationary, P streams)
                        for nh in range(NSW):
                            nc.tensor.matmul(
                                o_t[hx][:, nh * SW : (nh + 1) * SW],
                                lhsT=V[:, st, h, :],
                                rhs=pb[:, nh * SW : (nh + 1) * SW],
                                start=(st == 0),
                                stop=(st == TT - 1),
                            )
                        if st + 1 < TT:
                            s_cur[hx] = s_mm(hx, st + 1)
                # normalize into OT
                for hx in range(2):
                    # copy O_aug^T out of PSUM first so the bank frees fast
                    # (keeps PE streaming; the normalize chain below is long)
                    ou = pon.tile([65, TCH], F32, tag="ou", name="ou")
                    nc.vector.tensor_copy(ou, o_t[hx])
                    # 1/rowsum (plain DVE reciprocal: the custom-DVE approx
                    # op does not survive the neuronxcc path), then DRAM
                    # bounce -> stride-0 broadcast to 64 partitions
                    rr = pon.tile([1, TCH], F32, tag="rr", name="rr")
                    nc.vector.reciprocal(rr, ou[64:65, :])
                    rrd = pdram.tile([1, TCH], F32, tag="rrd", name="rrd")
                    nc.sync.dma_start(out=rrd, in_=rr)
                    rb = pon.tile([64, TCH], F32, tag="rb", name="rb")
                    nc.sync.dma_start(out=rb, in_=rrd.to_broadcast((64, TCH)))
                    nc.vector.tensor_mul(
                        OT[hx * 64 : (hx + 1) * 64, hp, th * TCH : (th + 1) * TCH],
                        ou[0:64, :],
                        rb,
                    )

        # ---- projection ----
        # weight-stationary: partial^T[c, t] = pw.T @ O^T; pw tiles load once
        # and stream 4x512 of O^T each. Host transposes the [C, T] output.
        for cp in range(CT // 2):      # pairs of c col-tiles
            ps_c = [
                pp.tile([P, T], F32, tag=o_tag(k), name="ps_c")
                for k in range(2)
            ]
            for kk in range(KT4):
                for k in range(2):
                    ct8 = 2 * cp + k
                    for tch in range(T // SW):
                        nc.tensor.matmul(
                            ps_c[k][:, tch * SW : (tch + 1) * SW],
                            lhsT=pw_s[:, kk, ct8 * P : (ct8 + 1) * P],
                            rhs=OT[:, kk, tch * SW : (tch + 1) * SW],
                            start=(kk == 0),
                            stop=(kk == KT4 - 1),
                        )
            for k in range(2):
                ot = pout.tile([P, T], F32, tag="ot", name="ot")
                nc.vector.tensor_copy(ot, ps_c[k])
                nc.sync.dma_start(
                    out=partial[(2 * cp + k) * P : (2 * cp + k + 1) * P, :], in_=ot
                )

    nc.compile()
    return nc


def shard_inputs(x, qkv_w, qkv_b, proj_w, proj_b, T=FULL_T):
    """Build the 8 per-core input maps (host-side layout prep)."""
    x = np.asarray(x, dtype=np.float32)
    qkv_w = np.asarray(qkv_w, dtype=np.float32)
    qkv_b = np.asarray(qkv_b, dtype=np.float32)
    proj_w = np.asarray(proj_w, dtype=np.float32)
    in_maps = []
    for c in range(N_CORES):
        b, g = divmod(c, 2)
        sl = slice(g * DG, (g + 1) * DG)
        wqg = qkv_w[:, 0 * C + g * DG : 0 * C + (g + 1) * DG]
        wkg = qkv_w[:, 1 * C + g * DG : 1 * C + (g + 1) * DG]
        wvg = qkv_w[:, 2 * C + g * DG : 2 * C + (g + 1) * DG]
        bqg = qkv_b[0 * C + g * DG : 0 * C + (g + 1) * DG]
        bkg = qkv_b[1 * C + g * DG : 1 * C + (g + 1) * DG]
        bvg = qkv_b[2 * C + g * DG : 2 * C + (g + 1) * DG]
        pwg = proj_w[sl, :]

        def arr_w(w):  # [C, DG] -> [128, C//128, DG]
            return np.ascontiguousarray(
                w.reshape(C // P, P, DG).transpose(1, 0, 2)
            ).astype(bf16)

        bqk = np.ascontiguousarray(
            np.concatenate(
                [bqg.reshape(DG // P, P).T, bkg.reshape(DG // P, P).T], axis=1
            )
        ).astype(np.float32)
        in_maps.append(
            {
                "x": np.ascontiguousarray(x[b, :T]).astype(bf16),
                "wq": arr_w(wqg),
                "wk": arr_w(wkg),
                "wv": arr_w(wvg),
                "bqk": bqk,
                "bv": np.ascontiguousarray(bvg[None, :]).astype(bf16),
                "pw": np.ascontiguousarray(
                    pwg.reshape(DG // P, P, C).transpose(1, 0, 2)
                ).astype(bf16),
            }
        )
    return in_maps


def combine_outputs(results, proj_b, T=FULL_T):
    proj_b = np.asarray(proj_b, dtype=np.float32)
    out = np.empty((N_CORES // 2, T, C), np.float32)
    for b in range(N_CORES // 2):
        out[b] = (
            results[2 * b]["partial"] + results[2 * b + 1]["partial"]
        ).T + proj_b
    return out


_NC_CACHE = {}


def _get_nc(T=FULL_T):
    if T not in _NC_CACHE:
        _NC_CACHE[T] = build_kernel(T)
    return _NC_CACHE[T]


def run(x, qkv_w, qkv_b, proj_w, proj_b, trace=False):
    nc = _get_nc()
    in_maps = shard_inputs(x, qkv_w, qkv_b, proj_w, proj_b)
    res = run_bass_kernel_spmd(nc, in_maps, list(range(N_CORES)), trace=trace)
    return combine_outputs(res.results, proj_b), res


def kernel(x, qkv_w, qkv_b, proj_w, proj_b):
    out, _ = run(x, qkv_w, qkv_b, proj_w, proj_b)
    return out


# revision 21
# speedup vs baseline: 1.4701x; 1.0082x over previous
"""Multi-head self-attention TRN2 kernel.

Full inputs -> shard over 8 NeuronCores as (batch b, head-group g):
core c = 2*b + g handles batch b and heads 8g..8g+7 (tensor parallel over
heads within a batch entry). Each core computes its heads' contribution to
the output projection; the host sums the two partials per batch and adds
proj bias.

Per-core pipeline (all matmuls bf16 with fp32 PSUM accumulation):
  x^T   via transposing DMA                      [C=1024, T=2048]
  Q^T,K^T = (w.T as lhsT) @ x^T  (+bias, DVE)    [512, T] col-major
  V     = (x^T as lhsT) @ wv (+bias via K=1 mm)  [T, 512] + ones col
  per head-pair hp, t-chunk of 1024, s-tile of 128:
    S^T chunk = K^T.T @ Q^T      (K=64 contraction, 2 heads row-tiled)
    P^T = exp(S^T/8)             ScalarE from PSUM, bf16 out, N=1024
    O  += P^T.T @ V_aug          (per 128-t subtile, N=65; col 64 = rowsum)
  normalize O by 1/rowsum (per-partition scalar), DMA-transpose -> O^T
  partial = O^T.T @ pw           [T, 1024] fp32 -> DRAM
"""

import numpy as np
import ml_dtypes
from contextlib import ExitStack

import concourse.bass as bass
import concourse.bacc as bacc
import concourse.tile as tile
from concourse import mybir
from concourse.bass_utils import run_bass_kernel_spmd

BF16 = mybir.dt.bfloat16
F32 = mybir.dt.float32
bf16 = ml_dtypes.bfloat16

P = 128
C = 1024          # hidden
HG = 8            # heads per core
D = 64            # head dim
DG = HG * D       # 512, per-core qkv width
N_CORES = 8
FULL_T = 2048
SCALE = D ** -0.5


def build_kernel(T=FULL_T):
    nc = bacc.Bacc(
        "TRN2", target_bir_lowering=False, debug=False, num_devices=N_CORES
    )
    x = nc.dram_tensor("x", [T, C], BF16, kind="ExternalInput").ap()
    wq = nc.dram_tensor("wq", [P, C // P, DG], BF16, kind="ExternalInput").ap()
    wk = nc.dram_tensor("wk", [P, C // P, DG], BF16, kind="ExternalInput").ap()
    wv = nc.dram_tensor("wv", [P, C // P, DG], BF16, kind="ExternalInput").ap()
    # cols 0..3 = q bias per col-tile, 4..7 = k bias
    bqk = nc.dram_tensor("bqk", [P, 8], F32, kind="ExternalInput").ap()
    bv = nc.dram_tensor("bv", [1, DG], BF16, kind="ExternalInput").ap()
    pw = nc.dram_tensor("pw", [P, DG // P, C], BF16, kind="ExternalInput").ap()
    partial = nc.dram_tensor("partial", [T, C], F32, kind="ExternalOutput").ap()

    CT = C // P           # 8 contraction tiles over hidden
    TT = T // P           # t/s tiles of 128
    TCH = min(1024, T)    # t chunk width for attention (exp granularity)
    NCH = T // TCH        # number of t chunks
    SW = min(512, TCH)    # matmul moving-dim width (PSUM bank limit)
    NSW = TCH // SW       # sub-chunks per chunk
    Q8 = TCH // P         # 128-t subtiles per chunk
    KT4 = DG // P         # 4 col-tiles of Q^T/K^T/O^T

    with tile.TileContext(nc) as tc, ExitStack() as ctx:
        sb = ctx.enter_context(tc.tile_pool(name="sb", bufs=1))
        pdram = ctx.enter_context(tc.tile_pool(name="pdram", bufs=4, space="DRAM"))
        pon = ctx.enter_context(tc.tile_pool(name="pon", bufs=3))
        ppb = ctx.enter_context(tc.tile_pool(name="ppb", bufs=16))
        pout = ctx.enter_context(tc.tile_pool(name="pout", bufs=2))
        pp = ctx.enter_context(tc.tile_pool(name="pp", bufs=1, space="PSUM"))

        def o_tag(hx):
            return ("oA", "oB")[hx]

        # persistent SBUF tensors
        xT = sb.tile([P, CT, T], BF16)
        wq_s = sb.tile([P, CT, DG], BF16)
        wk_s = sb.tile([P, CT, DG], BF16)
        wv_s = sb.tile([P, CT, DG], BF16)
        pw_s = sb.tile([P, KT4, C], BF16)
        bqk_s = sb.tile([P, 8], F32)
        bv_s = sb.tile([1, DG], BF16)
        ones_s = sb.tile([1, P], BF16)
        QT = sb.tile([P, KT4, T], BF16)
        KTt = sb.tile([P, KT4, T], BF16)
        V = sb.tile([P, TT, HG, 65], BF16)
        OT = sb.tile([P, KT4, T], BF16)

        # ---- loads ----
        # all transposing DMAs first (xbar-mode flips serialize against
        # normal copies), split across the two HWDGE queues
        nc.scalar.dma_start(out=wv_s, in_=wv)
        nc.scalar.dma_start(out=bv_s, in_=bv)
        nc.scalar.dma_start(out=wq_s, in_=wq)
        nc.scalar.dma_start(out=wk_s, in_=wk)
        nc.scalar.dma_start(out=bqk_s, in_=bqk)
        nc.scalar.dma_start(out=pw_s, in_=pw)
        for ct in range(CT):
            nc.sync.dma_start(
                out=xT[:, ct, :], in_=x[:, ct * P : (ct + 1) * P], transpose=True
            )
        nc.vector.memset(ones_s, 1.0)
        nc.vector.memset(V[:, :, :, 64:65], 1.0)

        # ---- V (natural layout, bias via K=1 matmul, ones col preset) ----
        for tt in range(TT):
            ps = pp.tile([P, DG], F32, tag=o_tag(tt % 2), name="psv")
            for ct in range(CT):
                nc.tensor.matmul(
                    ps,
                    lhsT=xT[:, ct, tt * P : (tt + 1) * P],
                    rhs=wv_s[:, ct, :],
                    start=(ct == 0),
                    stop=False,
                )
            nc.tensor.matmul(ps, lhsT=ones_s, rhs=bv_s, start=False, stop=True)
            nc.vector.tensor_copy(
                out=V[:, tt, :, 0:64],
                in_=ps.rearrange("p (h d) -> p h d", h=HG),
            )

        # ---- Q^T / K^T (emitted per col-tile, interleaved with attention
        # so the scheduler can hide later col-tiles under exp windows) ----
        def qk_tile(w_s, QKT, boff, i):
            for th in range(NCH):
                ps = pp.tile([P, TCH], F32, tag=o_tag((i + th) % 2), name="ps")
                for ct in range(CT):
                    for nh in range(NSW):
                        nc.tensor.matmul(
                            ps[:, nh * SW : (nh + 1) * SW],
                            lhsT=w_s[:, ct, i * P : (i + 1) * P],
                            rhs=xT[
                                :, ct, th * TCH + nh * SW : th * TCH + (nh + 1) * SW
                            ],
                            start=(ct == 0),
                            stop=(ct == CT - 1),
                        )
                nc.vector.tensor_scalar_add(
                    QKT[:, i, th * TCH : (th + 1) * TCH],
                    ps,
                    bqk_s[:, boff + i : boff + i + 1],
                )

        for _i in range(KT4):
            qk_tile(wk_s, KTt, 4, _i)
            qk_tile(wq_s, QT, 0, _i)

        # ---- attention ----
        # Software-pipelined: two head-streams (A/B) alternate on ScalarE;
        # while exp(B, st) runs, PE does PV(A, st) and S(A, st+1), so the
        # exp stream never waits. PSUM: sA, sB ([128, TCH] = 2 banks each,
        # bufs=1) + oA, oB ([65, TCH] accumulators) = 8 banks exactly.
        for hp in range(KT4):          # head pair = col-tile of QT/KT
            for th in range(NCH):      # t chunk of TCH
                o_t = [
                    pp.tile([65, TCH], F32, tag=o_tag(hx), name="o_t")
                    for hx in range(2)
                ]

                def s_mm(hx, st):
                    s_ps = pp.tile(
                        [P, TCH], F32, tag=("sA", "sB")[hx], name="s_ps"
                    )
                    pr = slice(hx * 64, (hx + 1) * 64)
                    for nh in range(NSW):
                        nc.tensor.matmul(
                            s_ps[:, nh * SW : (nh + 1) * SW],
                            lhsT=KTt[pr, hp, st * P : (st + 1) * P],
                            rhs=QT[
                                pr,
                                hp,
                                th * TCH + nh * SW : th * TCH + (nh + 1) * SW,
                            ],
                            start=True,
                            stop=True,
                        )
                    return s_ps

                s_cur = [s_mm(0, 0), s_mm(1, 0)]
                for st in range(TT):   # s tile of 128
                    for hx in range(2):
                        h = 2 * hp + hx
                        pb = ppb.tile([P, TCH], BF16, tag="p", name="pb")
                        nc.scalar.activation(
                            out=pb,
                            in_=s_cur[hx],
                            func=mybir.ActivationFunctionType.Exp,
                            scale=float(SCALE),
                        )
                        # O_aug^T += V_aug.T @ P^T (V stationary, P streams)
                        for nh in range(NSW):
                            nc.tensor.matmul(
                                o_t[hx][:, nh * SW : (nh + 1) * SW],
                                lhsT=V[:, st, h, :],
                                rhs=pb[:, nh * SW : (nh + 1) * SW],
                                start=(st == 0),
                                stop=(st == TT - 1),
                            )
                        if st + 1 < TT:
                            s_cur[hx] = s_mm(hx, st + 1)
                # normalize into OT
                for hx in range(2):
                    # copy O_aug^T out of PSUM first so the bank frees fast
                    # (keeps PE streaming; the normalize chain below is long)
                    ou = pon.tile([65, TCH], F32, tag="ou", name="ou")
                    nc.vector.tensor_copy(ou, o_t[hx])
                    # 1/rowsum (plain DVE reciprocal: the custom-DVE approx
                    # op does not survive the neuronxcc path), then DRAM
                    # bounce -> stride-0 broadcast to 64 partitions
                    rr = pon.tile([1, TCH], F32, tag="rr", name="rr")
                    nc.vector.reciprocal(rr, ou[64:65, :])
                    rrd = pdram.tile([1, TCH], F32, tag="rrd", name="rrd")
                    nc.sync.dma_start(out=rrd, in_=rr)
                    rb = pon.tile([64, TCH], F32, tag="rb", name="rb")
                    nc.sync.dma_start(out=rb, in_=rrd.to_broadcast((64, TCH)))
                    nc.vector.tensor_mul(
                        OT[hx * 64 : (hx + 1) * 64, hp, th * TCH : (th + 1) * TCH],
                        ou[0:64, :],
                        rb,
                    )

        # ---- projection ----
        for mt in range(TT):
            ps_p = pp.tile([P, C], F32, tag=o_tag(mt % 2), name="ps_p")
            for kk in range(KT4):
                for nh in range(C // 512):
                    nc.tensor.matmul(
                        ps_p[:, nh * 512 : (nh + 1) * 512],
                        lhsT=OT[:, kk, mt * P : (mt + 1) * P],
                        rhs=pw_s[:, kk, nh * 512 : (nh + 1) * 512],
                        start=(kk == 0),
                        stop=(kk == KT4 - 1),
                    )
            ot = pout.tile([P, C], F32, tag="ot", name="ot")
            nc.vector.tensor_copy(ot, ps_p)
            nc.sync.dma_start(out=partial[mt * P : (mt + 1) * P, :], in_=ot)

    nc.compile()
    return nc


def shard_inputs(x, qkv_w, qkv_b, proj_w, proj_b, T=FULL_T):
    """Build the 8 per-core input maps (host-side layout prep)."""
    x = np.asarray(x, dtype=np.float32)
    qkv_w = np.asarray(qkv_w, dtype=np.float32)
    qkv_b = np.asarray(qkv_b, dtype=np.float32)
    proj_w = np.asarray(proj_w, dtype=np.float32)
    in_maps = []
    for c in range(N_CORES):
        b, g = divmod(c, 2)
        sl = slice(g * DG, (g + 1) * DG)
        wqg = qkv_w[:, 0 * C + g * DG : 0 * C + (g + 1) * DG]
        wkg = qkv_w[:, 1 * C + g * DG : 1 * C + (g + 1) * DG]
        wvg = qkv_w[:, 2 * C + g * DG : 2 * C + (g + 1) * DG]
        bqg = qkv_b[0 * C + g * DG : 0 * C + (g + 1) * DG]
        bkg = qkv_b[1 * C + g * DG : 1 * C + (g + 1) * DG]
        bvg = qkv_b[2 * C + g * DG : 2 * C + (g + 1) * DG]
        pwg = proj_w[sl, :]

        def arr_w(w):  # [C, DG] -> [128, C//128, DG]
            return np.ascontiguousarray(
                w.reshape(C // P, P, DG).transpose(1, 0, 2)
            ).astype(bf16)

        bqk = np.ascontiguousarray(
            np.concatenate(
                [bqg.reshape(DG // P, P).T, bkg.reshape(DG // P, P).T], axis=1
            )
        ).astype(np.float32)
        in_maps.append(
            {
                "x": np.ascontiguousarray(x[b, :T]).astype(bf16),
                "wq": arr_w(wqg),
                "wk": arr_w(wkg),
                "wv": arr_w(wvg),
                "bqk": bqk,
                "bv": np.ascontiguousarray(bvg[None, :]).astype(bf16),
                "pw": np.ascontiguousarray(
                    pwg.reshape(DG // P, P, C).transpose(1, 0, 2)
                ).astype(bf16),
            }
        )
    return in_maps


def combine_outputs(results, proj_b, T=FULL_T):
    proj_b = np.asarray(proj_b, dtype=np.float32)
    out = np.empty((N_CORES // 2, T, C), np.float32)
    for b in range(N_CORES // 2):
        out[b] = (
            results[2 * b]["partial"] + results[2 * b + 1]["partial"] + proj_b
        )
    return out


_NC_CACHE = {}


def _get_nc(T=FULL_T):
    if T not in _NC_CACHE:
        _NC_CACHE[T] = build_kernel(T)
    return _NC_CACHE[T]


def run(x, qkv_w, qkv_b, proj_w, proj_b, trace=False):
    nc = _get_nc()
    in_maps = shard_inputs(x, qkv_w, qkv_b, proj_w, proj_b)
    res = run_bass_kernel_spmd(nc, in_maps, list(range(N_CORES)), trace=trace)
    return combine_outputs(res.results, proj_b), res


def kernel(x, qkv_w, qkv_b, proj_w, proj_b):
    out, _ = run(x, qkv_w, qkv_b, proj_w, proj_b)
    return out
